# revision 9
# baseline (speedup 1.0000x reference)
"""Trainium2 Bass kernel for masked pairwise-sigmoid GNN message passing.

Reference computation (per graph g with nodes i,j in [0,nv)):
    c = z @ Wc.T + bc ; y = z @ Wy.T + by          # [G, nv, H]
    s[g,i,j,:] = sigmoid(c[g,i,:] + y[g,j,:] + (m_i + m_j)*L - 2L)
    out[g,i,:] = sum_j s[g,i,j,:] / sum_j m[g,j]

Exact identity: with m in {0,1}, any pair with m_i==0 or m_j==0 has mask
term <= -1e10, so sigmoid underflows to exactly 0 in fp32.  Only active
nodes (m==1) contribute; for active pairs the mask term is 0.  The host
gathers active nodes per graph, the device computes the dense active x
active interaction, and the host scatters rows back.

Sharding: graphs sorted by active count, dealt round-robin to 8 cores in
4 slots; slot s padded to one global size P_s (multiple of 4) so one
SPMD program serves all cores.  Padding columns get a -1e5 additive mask
(sigmoid -> 0); padding rows are discarded on scatter.

Device layout keeps hidden dim on partitions end-to-end; the output is
stored channel-major [128, 2*NTOT] and the host transposes, eliminating
all PE transposes and per-slot PSUM evacuations of the baseline.  Engine
split per slot: pairwise add on DVE (slots 0,1) / GPSIMD (slots 2,3),
sigmoid (fp32 in, bf16 out) on ACT, a bf16 2x-mode halving add plus the
final reduce on DVE, one 1/n columnwise multiply, one output DMA.
Biases and the pad mask fold into the projection matmuls as extra
contraction rows; PSUM evacuations ride the ACT queue (Copy+Sigmoid
table loads both hide behind the input DMAs/matmuls).
"""

import numpy as np

import concourse.bass as bass
import concourse.mybir as mybir
import concourse.tile as tile
from concourse import bacc
from concourse.bass_utils import run_bass_kernel_spmd

F32 = mybir.dt.float32
BF16 = mybir.dt.bfloat16
N_CORES = 8
PAD_NEG = -1.0e5  # additive mask for padding columns; sigmoid(-1e5) == 0

# test.py reads this for profiling info after a traced run
_last_results = None
_program_cache = {}

# slot index -> engine for the pairwise add ("v" = vector, "g" = gpsimd)
ADD_ENGINES = ("v", "v", "g", "g")


def _ap(view, free_dims):
    """AP anchored at `view`'s base with custom free dims (stride, num)."""
    return bass.AP(
        tensor=view.tensor,
        offset=view.offset,
        ap=[list(view.ap[0])] + [[int(s), int(n)] for s, n in free_dims],
    )


def _build_program(P_list, PI_list, H):
    """P_list: per-slot j-extent (mult of 4); PI_list: per-slot i-extent."""
    NTOT = sum(P_list)
    assert H == 256
    nc = bacc.Bacc(None, target_bir_lowering=False)

    zT = nc.dram_tensor("zT", [128, 2 * NTOT], BF16, kind="ExternalInput")
    wcT = nc.dram_tensor("wcT", [128, 2 * H], BF16, kind="ExternalInput")
    wyT = nc.dram_tensor("wyT", [128, 2 * H], BF16, kind="ExternalInput")
    auxc = nc.dram_tensor("auxc", [1, H], BF16, kind="ExternalInput")
    auxy = nc.dram_tensor("auxy", [2, H], BF16, kind="ExternalInput")
    emr = nc.dram_tensor("emr", [1, NTOT], BF16, kind="ExternalInput")
    out = nc.dram_tensor("out", [128, 2 * NTOT], F32, kind="ExternalOutput")

    AT = mybir.ActivationFunctionType
    OP = mybir.AluOpType

    with tile.TileContext(nc) as tc:
        with (
            tc.tile_pool(name="singles", bufs=1) as singles,
            tc.tile_pool(name="ptp", bufs=2) as ptp,
            tc.tile_pool(name="stp", bufs=2) as stp,
            tc.tile_pool(name="hvp", bufs=2) as hvp,
            tc.tile_pool(name="oup", bufs=2) as oup,
            tc.tile_pool(name="psum", bufs=1, space="PSUM") as psum,
        ):
            # ---- input DMAs split by k-block across three queues so the
            # first projections' operands land as early as possible
            em_sb = singles.tile([2, NTOT], BF16, tag="em", name="em_sb")
            nc.gpsimd.memset(em_sb[0:1, :], 1.0)
            z_sb = singles.tile([128, 2 * NTOT], BF16, tag="z", name="z_sb")
            w_sb = {}
            for wname in ("c", "y"):
                w_sb[wname] = singles.tile(
                    [128, 2 * H], BF16, tag=f"w{wname}", name=f"w{wname}"
                )
            auxc_sb = singles.tile([1, H], BF16, tag="auxc", name="auxc_sb")
            auxy_sb = singles.tile([2, H], BF16, tag="auxy", name="auxy_sb")
            nc.sync.dma_start(out=z_sb[:, 0:NTOT], in_=zT[:, 0:NTOT])
            nc.scalar.dma_start(out=w_sb["y"][:, 0:H], in_=wyT[:, 0:H])
            nc.gpsimd.dma_start(out=w_sb["c"][:, 0:H], in_=wcT[:, 0:H])
            nc.sync.dma_start(out=em_sb[1:2, :], in_=emr[:])
            nc.scalar.dma_start(out=auxy_sb[:], in_=auxy[:])
            nc.gpsimd.dma_start(out=auxc_sb[:], in_=auxc[:])
            nc.sync.dma_start(out=z_sb[:, NTOT:2 * NTOT], in_=zT[:, NTOT:2 * NTOT])
            nc.scalar.dma_start(out=w_sb["y"][:, H:2 * H], in_=wyT[:, H:2 * H])
            nc.gpsimd.dma_start(out=w_sb["c"][:, H:2 * H], in_=wcT[:, H:2 * H])

            # ---- projections -> PSUM -> SBUF (ACT evacuation)
            ps_t = {}
            for wname, ob in (("y", 0), ("c", 0), ("y", 1), ("c", 1)):
                ps = psum.tile(
                    [128, NTOT], F32, tag=f"ps{wname}{ob}", name=f"ps{wname}{ob}"
                )
                for kb in range(2):
                    o0 = kb * H + ob * 128
                    nc.tensor.matmul(
                        ps[:],
                        lhsT=w_sb[wname][:, o0:o0 + 128],
                        rhs=z_sb[:, kb * NTOT:(kb + 1) * NTOT],
                        start=(kb == 0),
                        stop=False,
                    )
                if wname == "c":
                    nc.tensor.matmul(
                        ps[:], lhsT=auxc_sb[:, ob * 128:(ob + 1) * 128],
                        rhs=em_sb[0:1, :], start=False, stop=True,
                    )
                else:
                    nc.tensor.matmul(
                        ps[:], lhsT=auxy_sb[:, ob * 128:(ob + 1) * 128],
                        rhs=em_sb[:], start=False, stop=True,
                    )
                ps_t[wname, ob] = ps
            cy = singles.tile([128, 4 * NTOT], F32, tag="cy", name="cy")
            for wname, ob, idx in (("y", 0, 2), ("c", 0, 0), ("y", 1, 3), ("c", 1, 1)):
                nc.scalar.copy(cy[:, idx * NTOT:(idx + 1) * NTOT], ps_t[wname, ob][:])

            # ---- per-slot: fused-ob pairwise add -> sigmoid -> halve -> reduce
            offs = [0]
            for P in P_list[:-1]:
                offs.append(offs[-1] + P)
            for si, (P, PI) in enumerate(zip(P_list, PI_list)):
                col = offs[si]
                # in0[p, ob, i, j] = c'[p, ob, i]; in1[p, ob, i, j] = y'[p, ob, j]
                in0 = _ap(cy[:, col:col + PI], [(NTOT, 2), (1, PI), (0, P)])
                in1 = _ap(cy[:, 2 * NTOT + col:2 * NTOT + col + P],
                          [(NTOT, 2), (0, PI), (1, P)])
                pt = ptp.tile([128, 2, PI, P], F32, tag="pair", name="pair_t")
                nc.vector.tensor_tensor(out=pt[:], in0=in0, in1=in1, op=OP.add)
                st = stp.tile([128, 2, PI, P], BF16, tag="sig", name="sig_t")
                nc.scalar.activation(out=st[:], in_=pt[:], func=AT.Sigmoid)
                # bf16 halving add (2x packed mode; P % 4 == 0 keeps alignment)
                hw = P // 2
                hv = hvp.tile([128, 2, PI, hw], BF16, tag="hv", name="hv_t")
                nc.vector.tensor_tensor(
                    out=hv[:], in0=st[:, :, :, 0:hw], in1=st[:, :, :, hw:P],
                    op=OP.add,
                )
                red = oup.tile([128, 2, PI], F32, tag="red", name="red_t")
                nc.vector.reduce_sum(out=red[:], in_=hv[:], axis=mybir.AxisListType.X)
                nc.sync.dma_start(
                    out=_ap(out[0:128, col:col + PI], [(NTOT, 2), (1, PI)]),
                    in_=red[:],
                )

    nc.finalize()
    return nc


def kernel(num_graphs, nv, z, mask, Wc, bc, Wy, by):
    global _last_results
    G = int(num_graphs)
    NV = int(nv)
    z = np.ascontiguousarray(np.asarray(z, dtype=np.float32))
    mask = np.asarray(mask, dtype=np.float32).reshape(G, NV)
    Wc = np.asarray(Wc, dtype=np.float32)
    bc = np.asarray(bc, dtype=np.float32)
    Wy = np.asarray(Wy, dtype=np.float32)
    by = np.asarray(by, dtype=np.float32)
    H = z.shape[-1]
    zg = z.reshape(G, NV, H)

    out_full = np.zeros((G * NV, H), dtype=np.float32)

    # ---- host: active-node compaction & slot assignment ----
    act_idx = [np.nonzero(mask[g] > 0.5)[0] for g in range(G)]
    n_act = np.array([len(a) for a in act_idx])
    for g in range(G):
        if n_act[g] == 0:  # reference: 0/0 -> NaN for the whole graph
            out_full[g * NV:(g + 1) * NV, :] = np.nan

    order = np.argsort(-n_act, kind="stable")  # graphs by count, descending
    n_slots = (G + N_CORES - 1) // N_CORES
    assign = [[None] * n_slots for _ in range(N_CORES)]
    P_list = []
    for s in range(n_slots):
        ranks = order[s * N_CORES:(s + 1) * N_CORES]
        for c, g in enumerate(ranks):
            assign[c][s] = int(g)
        mx = max((int(n_act[g]) for g in ranks), default=0)
        P_list.append(max(4, (mx + 3) // 4 * 4))  # j-extent: multiple of 4
    PI_list = [max(1, max((int(n_act[g]) for g in order[s * N_CORES:(s + 1) * N_CORES]), default=1)) for s in range(n_slots)]
    offs = np.cumsum([0] + P_list[:-1]).tolist()
    NTOT = sum(P_list)

    # ---- host: per-core input staging ----
    import ml_dtypes
    def _interleave(wt):  # [256, F] -> [128, 2*F] with kb blocks side by side
        f = wt.shape[1]
        w2 = np.empty((128, 2 * f), dtype=ml_dtypes.bfloat16)
        w2[:, :f] = wt[:128]
        w2[:, f:] = wt[128:]
        return np.ascontiguousarray(w2)

    wcT = _interleave(Wc.T.astype(ml_dtypes.bfloat16))  # [h_in, o] blocks
    wyT = _interleave(Wy.T.astype(ml_dtypes.bfloat16))
    auxc = np.ascontiguousarray(bc.reshape(1, H).astype(ml_dtypes.bfloat16))
    auxy = np.ascontiguousarray(
        np.stack([by, np.ones(H, np.float32)]).astype(ml_dtypes.bfloat16)
    )

    in_maps = []
    for c in range(N_CORES):
        zT_act = np.zeros((H, NTOT), dtype=ml_dtypes.bfloat16)
        madd = np.full((1, NTOT), PAD_NEG, dtype=np.float32)
        for s in range(n_slots):
            g = assign[c][s]
            if g is None:
                continue
            n = int(n_act[g])
            if n == 0:
                continue
            o = int(offs[s])
            zT_act[:, o:o + n] = zg[g][act_idx[g]].T.astype(ml_dtypes.bfloat16)
            madd[0, o:o + n] = 0.0
        in_maps.append(
            {
                "zT": _interleave(zT_act),
                "wcT": wcT,
                "wyT": wyT,
                "auxc": auxc,
                "auxy": auxy,
                "emr": np.ascontiguousarray(madd.astype(ml_dtypes.bfloat16)),
            }
        )

    # ---- build + run ----
    key = (tuple(P_list), tuple(PI_list), H)
    nc = _program_cache.get(key)
    if nc is None:
        nc = _build_program(P_list, PI_list, H)
        _program_cache[key] = nc
    res = run_bass_kernel_spmd(nc, in_maps, list(range(N_CORES)))
    _last_results = res

    # ---- host: scatter back (device output is [h1, (ob, col)]-major) ----
    for c in range(N_CORES):
        oc = res.results[c]["out"].reshape(128, 2, NTOT)  # [h1, ob, col]
        for s in range(n_slots):
            g = assign[c][s]
            if g is None:
                continue
            n = int(n_act[g])
            if n == 0:
                continue
            o = int(offs[s])
            blk = oc[:, :, o:o + n]  # [128, 2, n] (unscaled sums)
            out_full[g * NV + act_idx[g], :] = (
                blk.transpose(2, 1, 0).reshape(n, H)
                * (np.float32(1.0) / np.float32(n))
            )
    return out_full
